# revision 31
# baseline (speedup 1.0000x reference)
"""Trainium2 Bass kernel for nn_CombineLoss_13477607375450.

Strategy: pure data-parallel over the batch dim (B=512 -> 64 per core x 8
cores). The only heavy inputs actually read by the loss are three [B,H,W]
f32 slices: cams1[idx,:,1], cams2[idx,:,1], cams1[1-idx,:,1] (~9.6 MB per
core). Each core:
  - streams its three [128, 6272] slabs (batch-halves on partitions),
  - DVE subtract + ACT Square-with-accum -> per-partition squared-diff sums,
  - computes the per-sample CE / weighting terms from the (tiny) preds on
    device,
  - reduces everything to one scalar partial via a PE dot product.
The host sums the 8 partial scalars (the "all-reduce").
"""

import os

import numpy as np

# ---- problem constants (hardcoded per task contract) ----
B = 512
H = W = 112
HW = H * W            # 12544
NCORES = 8
BPC = B // NCORES     # 64 batches per core
P = 128               # SBUF partitions; 2 half-rows per batch
HALF = HW // 2        # 6272 floats per partition
# many small chunks: Tile allows ~8 DMAs in flight, so small chunks stream
# through the queue; tapered front fills the DMA queue fast, tapered tail
# keeps the post-DMA compute chain tiny
CHUNKS = [784] * 7 + [560, 224]
assert sum(CHUNKS) == HALF

_NC_CACHE = {}


def _build_nc():
    import concourse.bacc as bacc
    import concourse.tile as tile
    from concourse import mybir

    import bass_rust
    from concourse.hw_specs import get_activation_tables

    f32 = mybir.dt.float32
    AF = mybir.ActivationFunctionType
    OP = mybir.AluOpType
    AX = mybir.AxisListType

    nc = bacc.Bacc("TRN2", target_bir_lowering=False, debug=False,
                   num_devices=NCORES)
    act_set_id = list(get_activation_tables("gen3").keys()).index(
        "natural_log_exp_and_others")
    # a/b/c slabs interleaved at chunk granularity: one DMA per chunk
    abc = nc.dram_tensor("abc", [P, 3 * HALF], f32, kind="ExternalInput").ap()
    small = nc.dram_tensor("small", [P, 9], f32, kind="ExternalInput").ap()
    outp = nc.dram_tensor("out", [1, 1], f32, kind="ExternalOutput").ap()

    with tile.TileContext(nc) as tc:
        with (
            tc.tile_pool(name="big", bufs=6) as big,
            tc.tile_pool(name="sm", bufs=1) as sm,
            tc.tile_pool(name="ps", bufs=1, space="PSUM") as ps,
        ):
            # Load the one ACT function table (Exp/Ln/Square) up front so it
            # overlaps the input DMA instead of stalling the first ACTIVATE.
            nc.scalar.add_instruction(bass_rust.InstLoadActFuncSet(
                name=nc.get_next_instruction_name(),
                engine=mybir.EngineType.Activation,
                act_func_set_id=act_set_id,
            ))

            # small preds go via the idle SWDGE queue so the Sync HWDGE ring's
            # first issue is chunk0's bulk transfer
            smt = sm.tile([P, 9], f32)
            nc.gpsimd.dma_start(out=smt, in_=small)
            ones = sm.tile([P, 1], f32)
            nc.vector.memset(ones, 1.0)

            NCHUNK = len(CHUNKS)
            er_parts = sm.tile([P, NCHUNK], f32)
            sp_parts = sm.tile([P, NCHUNK], f32)

            # ---- tiny per-sample math (runs early, off the critical path) ----
            # columns of smt: p1[0:2] p1o[2:4] p2[4:6] pb[6:8] yf[8:9]
            p1 = smt[:, 0:2]
            p1o = smt[:, 2:4]
            p2 = smt[:, 4:6]
            pb = smt[:, 6:8]
            yf = smt[:, 8:9]

            def lse2(ps_ap, tag):
                """logsumexp over the 2-class free dim; also returns d = x1-x0."""
                mx = sm.tile([P, 1], f32, tag=f"mx_{tag}")
                nc.vector.reduce_max(mx, ps_ap, axis=AX.X)
                dd = sm.tile([P, 1], f32, tag=f"dd_{tag}")
                nc.vector.tensor_sub(dd, ps_ap[:, 1:2], ps_ap[:, 0:1])
                nad = sm.tile([P, 1], f32, tag=f"nad_{tag}")
                nc.vector.tensor_scalar_mul(nad, dd, -1.0)
                nc.vector.tensor_tensor(out=nad, in0=dd, in1=nad, op=OP.min)
                # softplus(nad) = ln(exp(nad) + 1); no Softplus table on TRN2
                spt = sm.tile([P, 1], f32, tag=f"sp_{tag}")
                nc.scalar.activation(out=spt, in_=nad, func=AF.Exp)
                nc.scalar.activation(out=spt, in_=spt, func=AF.Ln, bias=1.0)
                ls = sm.tile([P, 1], f32, tag=f"ls_{tag}")
                nc.vector.tensor_add(ls, mx, spt)
                return ls, dd

            ls1, d1 = lse2(p1, "p1")
            ls2_, d2 = lse2(p2, "p2")
            lsb, _ = lse2(pb, "pb")

            # ce1 = ls1 - (p1_0 + yf*d1); ce2 likewise
            sel1 = sm.tile([P, 1], f32)
            nc.vector.tensor_mul(sel1, yf, d1)
            nc.vector.tensor_add(sel1, p1[:, 0:1], sel1)
            ce1 = sm.tile([P, 1], f32)
            nc.vector.tensor_sub(ce1, ls1, sel1)

            sel2 = sm.tile([P, 1], f32)
            nc.vector.tensor_mul(sel2, yf, d2)
            nc.vector.tensor_add(sel2, p2[:, 0:1], sel2)
            ce2 = sm.tile([P, 1], f32)
            nc.vector.tensor_sub(ce2, ls2_, sel2)

            q = sm.tile([P, 1], f32)          # q = 2*(ce + ce_back)
            nc.vector.tensor_add(q, ce1, ce2)
            cebr = sm.tile([P, 1], f32)
            nc.vector.tensor_sub(cebr, lsb, pb[:, 0:1])
            nc.vector.tensor_mul(cebr, cebr, yf)
            nc.vector.tensor_add(q, q, cebr)

            cur = sm.tile([P, 1], f32)
            nc.vector.tensor_tensor(out=cur, in0=p1[:, 1:2], in1=p1[:, 0:1],
                                    op=OP.is_gt)
            flag = sm.tile([P, 1], f32)
            nc.vector.tensor_tensor(out=flag, in0=p1o[:, 1:2], in1=p1o[:, 0:1],
                                    op=OP.is_gt)
            neq = sm.tile([P, 1], f32)
            nc.vector.tensor_tensor(out=neq, in0=cur, in1=flag, op=OP.not_equal)
            sameflag = sm.tile([P, 1], f32)
            nc.vector.tensor_scalar(out=sameflag, in0=neq, scalar1=-1.0,
                                    scalar2=1.0, op0=OP.mult, op1=OP.add)

            pm = sm.tile([P, 1], f32)
            nc.vector.tensor_sub(pm, p1[:, 1:2], ls1)
            prob1 = sm.tile([P, 1], f32)
            nc.scalar.activation(out=prob1, in_=pm, func=AF.Exp)

            om = sm.tile([P, 1], f32)         # 1 - cur_pred
            nc.vector.tensor_scalar(out=om, in0=cur, scalar1=-1.0,
                                    scalar2=1.0, op0=OP.mult, op1=OP.add)
            cond = sm.tile([P, 1], f32)
            nc.vector.tensor_mul(cond, neq, om)
            nc.vector.tensor_mul(cond, cond, yf)
            p1m1 = sm.tile([P, 1], f32)
            nc.vector.tensor_scalar_add(p1m1, prob1, -1.0)
            wv = sm.tile([P, 1], f32)
            nc.vector.tensor_mul(wv, cond, p1m1)
            nc.vector.tensor_scalar_add(wv, wv, 1.0)

            # pre-scaled coefficient vectors (off the critical path); the
            # final scalar is accumulated in PSUM via incremental matmuls
            cepart = sm.tile([P, 1], f32)     # w*(ce+ce_back)/(2B) per half-row
            nc.vector.scalar_tensor_tensor(out=cepart, in0=q,
                                           scalar=1.0 / (4 * B), in1=wv,
                                           op0=OP.mult, op1=OP.mult)
            coef_er = sm.tile([P, 1], f32)    # w*yf/(B*HW)
            nc.vector.scalar_tensor_tensor(out=coef_er, in0=wv,
                                           scalar=1.0 / (B * HW), in1=yf,
                                           op0=OP.mult, op1=OP.mult)
            coef_sp = sm.tile([P, 1], f32)    # yf*same/(B*HW)
            nc.vector.scalar_tensor_tensor(out=coef_sp, in0=sameflag,
                                           scalar=1.0 / (B * HW), in1=yf,
                                           op0=OP.mult, op1=OP.mult)

            # PSUM scalar accumulator: cepart term first, then one matmul per
            # landed chunk accumulator
            pt = ps.tile([1, 1], f32)
            nc.tensor.matmul(out=pt, lhsT=cepart, rhs=ones, start=True,
                             stop=False)

            # ---- heavy streaming part ----
            off = 0
            for ci, cf in enumerate(CHUNKS):
                last = ci == len(CHUNKS) - 1
                abct = big.tile([P, 3 * cf], f32, tag="abct")
                nc.sync.dma_start(out=abct, in_=abc[:, 3 * off:3 * (off + cf)])
                off += cf
                at = abct[:, 0:cf]
                bt = abct[:, cf:2 * cf]
                ct = abct[:, 2 * cf:3 * cf]
                d = big.tile([P, cf], f32, tag="d")
                nc.vector.tensor_sub(d, at, bt)
                if last:
                    # keep the tail off the congested ACT queue: DVE fused
                    # square+row-sum (custom uop, no accumulator-read step)
                    nc.vector.affine_mul_reduce(
                        out=d, accum_out=er_parts[:, ci:ci + 1],
                        in0=d, in1=d, scale=1.0, bias=0.0)
                else:
                    nc.scalar.activation(out=d, in_=d, func=AF.Square,
                                         accum_out=er_parts[:, ci:ci + 1])
                nc.tensor.matmul(out=pt, lhsT=coef_er,
                                 rhs=er_parts[:, ci:ci + 1], start=False,
                                 stop=False)
                e = big.tile([P, cf], f32, tag="e")
                nc.vector.tensor_sub(e, at, ct)
                if last:
                    nc.vector.affine_mul_reduce(
                        out=e, accum_out=sp_parts[:, ci:ci + 1],
                        in0=e, in1=e, scale=1.0, bias=0.0)
                else:
                    nc.scalar.activation(out=e, in_=e, func=AF.Square,
                                         accum_out=sp_parts[:, ci:ci + 1])
                nc.tensor.matmul(out=pt, lhsT=coef_sp,
                                 rhs=sp_parts[:, ci:ci + 1], start=False,
                                 stop=last)

            res_sb = sm.tile([1, 1], f32)
            nc.vector.tensor_copy(res_sb, pt)
            nc.sync.dma_start(out=outp, in_=res_sb)

    nc.compile()
    return nc


def _get_nc():
    if "nc" not in _NC_CACHE:
        _NC_CACHE["nc"] = _build_nc()
    return _NC_CACHE["nc"]


def kernel(preds1, cams1, preds1_back, preds2, cams2, y, index):
    from concourse.bass_utils import run_bass_kernel_spmd

    idx = int(np.asarray(index))
    preds1 = np.asarray(preds1, dtype=np.float32)
    preds1_back = np.asarray(preds1_back, dtype=np.float32)
    preds2 = np.asarray(preds2, dtype=np.float32)
    cams1 = np.asarray(cams1, dtype=np.float32)
    cams2 = np.asarray(cams2, dtype=np.float32)
    yf = np.asarray(y).astype(np.float32).reshape(B, 1)

    nc = _get_nc()

    in_maps = []
    for k in range(NCORES):
        s = slice(k * BPC, (k + 1) * BPC)
        a = cams1[idx, s, 1].reshape(P, HALF)
        b = cams2[idx, s, 1].reshape(P, HALF)
        c = cams1[1 - idx, s, 1].reshape(P, HALF)
        abc = np.empty((P, 3 * HALF), dtype=np.float32)
        off = 0
        for cf in CHUNKS:
            sl = slice(off, off + cf)
            abc[:, 3 * off:3 * off + cf] = a[:, sl]
            abc[:, 3 * off + cf:3 * off + 2 * cf] = b[:, sl]
            abc[:, 3 * off + 2 * cf:3 * off + 3 * cf] = c[:, sl]
            off += cf
        sm_host = np.concatenate(
            [preds1[idx, s], preds1[1 - idx, s], preds2[idx, s],
             preds1_back[idx, s], yf[s]], axis=1)          # [64, 9]
        sm_host = np.repeat(sm_host, 2, axis=0)            # [128, 9]
        in_maps.append({"abc": abc, "small": np.ascontiguousarray(sm_host)})

    trace = bool(int(os.environ.get("KERNEL_TRACE", "0")))
    res = run_bass_kernel_spmd(nc, in_maps, core_ids=list(range(NCORES)),
                               trace=trace)
    kernel.last_exec_time_ns = res.exec_time_ns
    total = sum(float(res.results[k]["out"][0, 0]) for k in range(NCORES))
    return np.array(total, dtype=np.float32)


kernel.last_exec_time_ns = None


# revision 32
# speedup vs baseline: 1.2873x; 1.2873x over previous
"""Trainium2 Bass kernel for nn_CombineLoss_13477607375450.

Strategy: data-parallel over the batch dim (B=512 across 8 cores), with
label-masked shipping: every CAM term of the loss (er, same_loss) is
multiplied by y in {0,1}, so batches with y=0 never touch the CAM tensors.
The host ships CAM slabs only for y=1 batches (~half the bytes), compacted
into 32 slots/core in a quarter-row layout (batch -> 4 partitions x 3136
floats). Per-sample CE/weight math runs on device for all batches; shipped
slots carry their own preds rows so the device derives every coefficient
itself. Zero-padded slots get yf=0 -> zero coefficients. A full-ship kernel
remains as fallback if more than 256 batches have y=1.
The host sums the 8 per-core partial scalars (the "all-reduce").
"""

import os

import numpy as np

# ---- problem constants (hardcoded per task contract) ----
B = 512
H = W = 112
HW = H * W            # 12544
NCORES = 8
BPC = B // NCORES     # 64 batches per core
P = 128               # SBUF partitions
HALF = HW // 2        # 6272; full path: 2 half-rows per batch
QROW = HW // 4        # 3136; masked path: 4 quarter-rows per batch
SLOTS = 32            # masked path: CAM batches per core (4*32 = 128 parts)
CAP = NCORES * SLOTS  # 256 y=1 batches max for the masked path

# chunking along the free dim; tapered tail keeps the post-DMA chain tiny
CHUNKS_FULL = [784] * 7 + [560, 224]
assert sum(CHUNKS_FULL) == HALF
CHUNKS_MASK = [560] * 5 + [336]
assert sum(CHUNKS_MASK) == QROW

_NC_CACHE = {}


def _build_nc(masked):
    import concourse.bacc as bacc
    import concourse.tile as tile
    from concourse import mybir

    import bass_rust
    from concourse.hw_specs import get_activation_tables

    f32 = mybir.dt.float32
    AF = mybir.ActivationFunctionType
    OP = mybir.AluOpType
    AX = mybir.AxisListType

    chunks = CHUNKS_MASK if masked else CHUNKS_FULL
    row = QROW if masked else HALF

    nc = bacc.Bacc("TRN2", target_bir_lowering=False, debug=False,
                   num_devices=NCORES)
    act_set_id = list(get_activation_tables("gen3").keys()).index(
        "natural_log_exp_and_others")
    # a/b/c slabs interleaved at chunk granularity: one DMA per chunk
    abc = nc.dram_tensor("abc", [P, 3 * row], f32, kind="ExternalInput").ap()
    small = nc.dram_tensor("small", [P, 9], f32, kind="ExternalInput").ap()
    if masked:
        small_cam = nc.dram_tensor("small_cam", [P, 9], f32,
                                   kind="ExternalInput").ap()
    outp = nc.dram_tensor("out", [1, 1], f32, kind="ExternalOutput").ap()

    with tile.TileContext(nc) as tc:
        with (
            tc.tile_pool(name="big", bufs=6) as big,
            tc.tile_pool(name="sm", bufs=1) as sm,
            tc.tile_pool(name="ps", bufs=1, space="PSUM") as ps,
        ):
            # Load the one ACT function table (Exp/Ln/Square) up front so it
            # overlaps the input DMA instead of stalling the first ACTIVATE.
            nc.scalar.add_instruction(bass_rust.InstLoadActFuncSet(
                name=nc.get_next_instruction_name(),
                engine=mybir.EngineType.Activation,
                act_func_set_id=act_set_id,
            ))

            # small preds go via the idle SWDGE queue so the Sync HWDGE ring's
            # first issue is chunk0's bulk transfer
            smt = sm.tile([P, 9], f32)
            nc.gpsimd.dma_start(out=smt, in_=small)
            if masked:
                smc = sm.tile([P, 9], f32)
                nc.gpsimd.dma_start(out=smc, in_=small_cam)
            ones = sm.tile([P, 1], f32)
            nc.vector.memset(ones, 1.0)

            NCHUNK = len(chunks)
            er_parts = sm.tile([P, NCHUNK], f32)
            sp_parts = sm.tile([P, NCHUNK], f32)

            def lse2(ps_ap, tag):
                """logsumexp over the 2-class free dim; also returns d = x1-x0."""
                mx = sm.tile([P, 1], f32, tag=f"mx_{tag}")
                nc.vector.reduce_max(mx, ps_ap, axis=AX.X)
                dd = sm.tile([P, 1], f32, tag=f"dd_{tag}")
                nc.vector.tensor_sub(dd, ps_ap[:, 1:2], ps_ap[:, 0:1])
                nad = sm.tile([P, 1], f32, tag=f"nad_{tag}")
                nc.vector.tensor_scalar_mul(nad, dd, -1.0)
                nc.vector.tensor_tensor(out=nad, in0=dd, in1=nad, op=OP.min)
                # softplus(nad) = ln(exp(nad) + 1); no Softplus table on TRN2
                spt = sm.tile([P, 1], f32, tag=f"sp_{tag}")
                nc.scalar.activation(out=spt, in_=nad, func=AF.Exp)
                nc.scalar.activation(out=spt, in_=spt, func=AF.Ln, bias=1.0)
                ls = sm.tile([P, 1], f32, tag=f"ls_{tag}")
                nc.vector.tensor_add(ls, mx, spt)
                return ls, dd

            def weight_chain(p1, p1o, yf, tag):
                """w = where(cond, softmax(p1)[1], 1) and same flag, per row."""
                ls1, d1 = lse2(p1, f"p1_{tag}")
                pm = sm.tile([P, 1], f32, tag=f"pm_{tag}")
                nc.vector.tensor_sub(pm, p1[:, 1:2], ls1)
                prob1 = sm.tile([P, 1], f32, tag=f"pr_{tag}")
                nc.scalar.activation(out=prob1, in_=pm, func=AF.Exp)
                cur = sm.tile([P, 1], f32, tag=f"cur_{tag}")
                nc.vector.tensor_tensor(out=cur, in0=p1[:, 1:2],
                                        in1=p1[:, 0:1], op=OP.is_gt)
                flag = sm.tile([P, 1], f32, tag=f"flag_{tag}")
                nc.vector.tensor_tensor(out=flag, in0=p1o[:, 1:2],
                                        in1=p1o[:, 0:1], op=OP.is_gt)
                neq = sm.tile([P, 1], f32, tag=f"neq_{tag}")
                nc.vector.tensor_tensor(out=neq, in0=cur, in1=flag,
                                        op=OP.not_equal)
                sameflag = sm.tile([P, 1], f32, tag=f"same_{tag}")
                nc.vector.tensor_scalar(out=sameflag, in0=neq, scalar1=-1.0,
                                        scalar2=1.0, op0=OP.mult, op1=OP.add)
                om = sm.tile([P, 1], f32, tag=f"om_{tag}")
                nc.vector.tensor_scalar(out=om, in0=cur, scalar1=-1.0,
                                        scalar2=1.0, op0=OP.mult, op1=OP.add)
                cond = sm.tile([P, 1], f32, tag=f"cond_{tag}")
                nc.vector.tensor_mul(cond, neq, om)
                nc.vector.tensor_mul(cond, cond, yf)
                p1m1 = sm.tile([P, 1], f32, tag=f"p1m1_{tag}")
                nc.vector.tensor_scalar_add(p1m1, prob1, -1.0)
                wv = sm.tile([P, 1], f32, tag=f"wv_{tag}")
                nc.vector.tensor_mul(wv, cond, p1m1)
                nc.vector.tensor_scalar_add(wv, wv, 1.0)
                return wv, sameflag, ls1, d1

            # ---- CE path: per-sample losses for this core's 64 batches ----
            # columns of smt: p1[0:2] p1o[2:4] p2[4:6] pb[6:8] yf[8:9]
            p1 = smt[:, 0:2]
            p2 = smt[:, 4:6]
            pb = smt[:, 6:8]
            yf = smt[:, 8:9]
            wv, sameflag, ls1, d1 = weight_chain(p1, smt[:, 2:4], yf, "ce")
            ls2_, d2 = lse2(p2, "p2")
            lsb, _ = lse2(pb, "pb")

            sel1 = sm.tile([P, 1], f32)
            nc.vector.tensor_mul(sel1, yf, d1)
            nc.vector.tensor_add(sel1, p1[:, 0:1], sel1)
            ce1 = sm.tile([P, 1], f32)
            nc.vector.tensor_sub(ce1, ls1, sel1)

            sel2 = sm.tile([P, 1], f32)
            nc.vector.tensor_mul(sel2, yf, d2)
            nc.vector.tensor_add(sel2, p2[:, 0:1], sel2)
            ce2 = sm.tile([P, 1], f32)
            nc.vector.tensor_sub(ce2, ls2_, sel2)

            q = sm.tile([P, 1], f32)          # q = 2*(ce + ce_back)
            nc.vector.tensor_add(q, ce1, ce2)
            cebr = sm.tile([P, 1], f32)
            nc.vector.tensor_sub(cebr, lsb, pb[:, 0:1])
            nc.vector.tensor_mul(cebr, cebr, yf)
            nc.vector.tensor_add(q, q, cebr)

            cepart = sm.tile([P, 1], f32)     # w*(ce+ce_back)/(2B) per half-row
            nc.vector.scalar_tensor_tensor(out=cepart, in0=q,
                                           scalar=1.0 / (4 * B), in1=wv,
                                           op0=OP.mult, op1=OP.mult)

            # ---- CAM-path coefficients ----
            dup = 4 if masked else 2
            if masked:
                yfc = smc[:, 8:9]
                wc, samec, _, _ = weight_chain(smc[:, 0:2], smc[:, 2:4],
                                               yfc, "cam")
            else:
                yfc, wc, samec = yf, wv, sameflag
            coef_er = sm.tile([P, 1], f32)    # w*yf/(B*HW)
            nc.vector.scalar_tensor_tensor(out=coef_er, in0=wc,
                                           scalar=1.0 / (B * HW), in1=yfc,
                                           op0=OP.mult, op1=OP.mult)
            coef_sp = sm.tile([P, 1], f32)    # yf*same/(B*HW)
            nc.vector.scalar_tensor_tensor(out=coef_sp, in0=samec,
                                           scalar=1.0 / (B * HW), in1=yfc,
                                           op0=OP.mult, op1=OP.mult)

            # PSUM scalar accumulator: cepart term first, then one matmul per
            # landed chunk accumulator
            pt = ps.tile([1, 1], f32)
            nc.tensor.matmul(out=pt, lhsT=cepart, rhs=ones, start=True,
                             stop=False)

            # ---- heavy streaming part ----
            off = 0
            for ci, cf in enumerate(chunks):
                last = ci == len(chunks) - 1
                abct = big.tile([P, 3 * cf], f32, tag="abct")
                nc.sync.dma_start(out=abct, in_=abc[:, 3 * off:3 * (off + cf)])
                off += cf
                at = abct[:, 0:cf]
                bt = abct[:, cf:2 * cf]
                ct = abct[:, 2 * cf:3 * cf]
                d = big.tile([P, cf], f32, tag="d")
                nc.vector.tensor_sub(d, at, bt)
                if last:
                    # keep the tail off the congested ACT queue: DVE fused
                    # square+row-sum (custom uop, no accumulator-read step)
                    nc.vector.affine_mul_reduce(
                        out=d, accum_out=er_parts[:, ci:ci + 1],
                        in0=d, in1=d, scale=1.0, bias=0.0)
                else:
                    nc.scalar.activation(out=d, in_=d, func=AF.Square,
                                         accum_out=er_parts[:, ci:ci + 1])
                nc.tensor.matmul(out=pt, lhsT=coef_er,
                                 rhs=er_parts[:, ci:ci + 1], start=False,
                                 stop=False)
                e = big.tile([P, cf], f32, tag="e")
                nc.vector.tensor_sub(e, at, ct)
                if last:
                    nc.vector.affine_mul_reduce(
                        out=e, accum_out=sp_parts[:, ci:ci + 1],
                        in0=e, in1=e, scale=1.0, bias=0.0)
                else:
                    nc.scalar.activation(out=e, in_=e, func=AF.Square,
                                         accum_out=sp_parts[:, ci:ci + 1])
                nc.tensor.matmul(out=pt, lhsT=coef_sp,
                                 rhs=sp_parts[:, ci:ci + 1], start=False,
                                 stop=last)

            res_sb = sm.tile([1, 1], f32)
            nc.vector.tensor_copy(res_sb, pt)
            nc.sync.dma_start(out=outp, in_=res_sb)

    nc.compile()
    return nc


def _get_nc(masked):
    key = "mask" if masked else "full"
    if key not in _NC_CACHE:
        _NC_CACHE[key] = _build_nc(masked)
    return _NC_CACHE[key]


def _interleave(a, b, c, chunks):
    """[P, row] x3 -> [P, 3*row] with a/b/c interleaved per chunk."""
    row = a.shape[1]
    abc = np.empty((P, 3 * row), dtype=np.float32)
    off = 0
    for cf in chunks:
        sl = slice(off, off + cf)
        abc[:, 3 * off:3 * off + cf] = a[:, sl]
        abc[:, 3 * off + cf:3 * off + 2 * cf] = b[:, sl]
        abc[:, 3 * off + 2 * cf:3 * off + 3 * cf] = c[:, sl]
        off += cf
    return abc


def kernel(preds1, cams1, preds1_back, preds2, cams2, y, index):
    from concourse.bass_utils import run_bass_kernel_spmd

    idx = int(np.asarray(index))
    preds1 = np.asarray(preds1, dtype=np.float32)
    preds1_back = np.asarray(preds1_back, dtype=np.float32)
    preds2 = np.asarray(preds2, dtype=np.float32)
    cams1 = np.asarray(cams1, dtype=np.float32)
    cams2 = np.asarray(cams2, dtype=np.float32)
    yi = np.asarray(y).astype(np.int64).reshape(B)
    yf = yi.astype(np.float32).reshape(B, 1)

    sel = np.flatnonzero(yi == 1)
    masked = len(sel) <= CAP
    nc = _get_nc(masked)

    in_maps = []
    for k in range(NCORES):
        s = slice(k * BPC, (k + 1) * BPC)
        sm_host = np.concatenate(
            [preds1[idx, s], preds1[1 - idx, s], preds2[idx, s],
             preds1_back[idx, s], yf[s]], axis=1)          # [64, 9]
        im = {"small": np.ascontiguousarray(
            np.repeat(sm_host, 2, axis=0))}                # [128, 9]

        if masked:
            sel_k = sel[k * SLOTS:(k + 1) * SLOTS]
            nk = len(sel_k)
            a = np.zeros((SLOTS, HW), dtype=np.float32)
            b = np.zeros((SLOTS, HW), dtype=np.float32)
            c = np.zeros((SLOTS, HW), dtype=np.float32)
            a[:nk] = cams1[idx, sel_k, 1].reshape(nk, HW)
            b[:nk] = cams2[idx, sel_k, 1].reshape(nk, HW)
            c[:nk] = cams1[1 - idx, sel_k, 1].reshape(nk, HW)
            im["abc"] = _interleave(a.reshape(P, QROW), b.reshape(P, QROW),
                                    c.reshape(P, QROW), CHUNKS_MASK)
            sc = np.zeros((SLOTS, 9), dtype=np.float32)
            sc[:nk] = np.concatenate(
                [preds1[idx, sel_k], preds1[1 - idx, sel_k],
                 preds2[idx, sel_k], preds1_back[idx, sel_k],
                 yf[sel_k]], axis=1)
            im["small_cam"] = np.ascontiguousarray(np.repeat(sc, 4, axis=0))
        else:
            a = cams1[idx, s, 1].reshape(P, HALF)
            b = cams2[idx, s, 1].reshape(P, HALF)
            c = cams1[1 - idx, s, 1].reshape(P, HALF)
            im["abc"] = _interleave(a, b, c, CHUNKS_FULL)
        in_maps.append(im)

    trace = bool(int(os.environ.get("KERNEL_TRACE", "0")))
    res = run_bass_kernel_spmd(nc, in_maps, core_ids=list(range(NCORES)),
                               trace=trace)
    kernel.last_exec_time_ns = res.exec_time_ns
    total = sum(float(res.results[k]["out"][0, 0]) for k in range(NCORES))
    return np.array(total, dtype=np.float32)


kernel.last_exec_time_ns = None


# revision 35
# speedup vs baseline: 1.3108x; 1.0182x over previous
"""Trainium2 Bass kernel for nn_CombineLoss_13477607375450.

Strategy: data-parallel over the batch dim (B=512 across 8 cores), with
label-masked shipping: every CAM term of the loss (er, same_loss) is
multiplied by y in {0,1}, so batches with y=0 never touch the CAM tensors.
The host ships CAM slabs only for y=1 batches (~half the bytes), compacted
into 32 slots/core in a quarter-row layout (batch -> 4 partitions x 3136
floats). Per-sample CE/weight math runs on device for all batches; shipped
slots carry their own preds rows so the device derives every coefficient
itself. Zero-padded slots get yf=0 -> zero coefficients. A full-ship kernel
remains as fallback if more than 256 batches have y=1.
The host sums the 8 per-core partial scalars (the "all-reduce").
"""

import os

import numpy as np

# ---- problem constants (hardcoded per task contract) ----
B = 512
H = W = 112
HW = H * W            # 12544
NCORES = 8
BPC = B // NCORES     # 64 batches per core
P = 128               # SBUF partitions
HALF = HW // 2        # 6272; full path: 2 half-rows per batch
QROW = HW // 4        # 3136; masked path: 4 quarter-rows per batch
SLOTS = 32            # masked path: CAM batches per core (4*32 = 128 parts)
CAP = NCORES * SLOTS  # 256 y=1 batches max for the masked path

# chunking along the free dim; tapered tail keeps the post-DMA chain tiny
CHUNKS_FULL = [784] * 7 + [560, 224]
assert sum(CHUNKS_FULL) == HALF
CHUNKS_MASK = [560] * 5 + [336]
assert sum(CHUNKS_MASK) == QROW

_NC_CACHE = {}


def _build_nc(masked):
    import concourse.bacc as bacc
    import concourse.tile as tile
    from concourse import mybir

    import bass_rust
    from concourse.hw_specs import get_activation_tables

    f32 = mybir.dt.float32
    AF = mybir.ActivationFunctionType
    OP = mybir.AluOpType
    AX = mybir.AxisListType

    chunks = CHUNKS_MASK if masked else CHUNKS_FULL
    row = QROW if masked else HALF

    nc = bacc.Bacc("TRN2", target_bir_lowering=False, debug=False,
                   num_devices=NCORES)
    act_set_id = list(get_activation_tables("gen3").keys()).index(
        "natural_log_exp_and_others")
    # a/b/c slabs interleaved at chunk granularity: one DMA per chunk
    abc = nc.dram_tensor("abc", [P, 3 * row], f32, kind="ExternalInput").ap()
    small = nc.dram_tensor("small", [P, 9], f32, kind="ExternalInput").ap()
    if masked:
        small_cam = nc.dram_tensor("small_cam", [P, 9], f32,
                                   kind="ExternalInput").ap()
    outp = nc.dram_tensor("out", [1, 1], f32, kind="ExternalOutput").ap()

    with tile.TileContext(nc) as tc:
        with (
            tc.tile_pool(name="big", bufs=6) as big,
            tc.tile_pool(name="sm", bufs=1) as sm,
            tc.tile_pool(name="ps", bufs=1, space="PSUM") as ps,
        ):
            # Load the one ACT function table (Exp/Ln/Square) up front so it
            # overlaps the input DMA instead of stalling the first ACTIVATE.
            nc.scalar.add_instruction(bass_rust.InstLoadActFuncSet(
                name=nc.get_next_instruction_name(),
                engine=mybir.EngineType.Activation,
                act_func_set_id=act_set_id,
            ))

            # small preds go via the idle SWDGE queue so the Sync HWDGE ring's
            # first issue is chunk0's bulk transfer
            smt = sm.tile([P, 9], f32)
            nc.gpsimd.dma_start(out=smt, in_=small)
            if masked:
                smc = sm.tile([P, 9], f32)
                nc.gpsimd.dma_start(out=smc, in_=small_cam)
            ones = sm.tile([P, 1], f32)
            nc.vector.memset(ones, 1.0)

            NCHUNK = len(chunks)
            er_parts = sm.tile([P, NCHUNK], f32)
            sp_parts = sm.tile([P, NCHUNK], f32)

            def lse2(ps_ap, tag):
                """logsumexp over the 2-class free dim; also returns d = x1-x0."""
                mx = sm.tile([P, 1], f32, tag=f"mx_{tag}")
                nc.vector.reduce_max(mx, ps_ap, axis=AX.X)
                dd = sm.tile([P, 1], f32, tag=f"dd_{tag}")
                nc.vector.tensor_sub(dd, ps_ap[:, 1:2], ps_ap[:, 0:1])
                nad = sm.tile([P, 1], f32, tag=f"nad_{tag}")
                nc.vector.tensor_scalar_mul(nad, dd, -1.0)
                nc.vector.tensor_tensor(out=nad, in0=dd, in1=nad, op=OP.min)
                # softplus(nad) = ln(exp(nad) + 1); no Softplus table on TRN2
                spt = sm.tile([P, 1], f32, tag=f"sp_{tag}")
                nc.scalar.activation(out=spt, in_=nad, func=AF.Exp)
                nc.scalar.activation(out=spt, in_=spt, func=AF.Ln, bias=1.0)
                ls = sm.tile([P, 1], f32, tag=f"ls_{tag}")
                nc.vector.tensor_add(ls, mx, spt)
                return ls, dd

            def weight_chain(p1, p1o, yf, tag):
                """w = where(cond, softmax(p1)[1], 1) and same flag, per row."""
                ls1, d1 = lse2(p1, f"p1_{tag}")
                pm = sm.tile([P, 1], f32, tag=f"pm_{tag}")
                nc.vector.tensor_sub(pm, p1[:, 1:2], ls1)
                prob1 = sm.tile([P, 1], f32, tag=f"pr_{tag}")
                nc.scalar.activation(out=prob1, in_=pm, func=AF.Exp)
                cur = sm.tile([P, 1], f32, tag=f"cur_{tag}")
                nc.vector.tensor_tensor(out=cur, in0=p1[:, 1:2],
                                        in1=p1[:, 0:1], op=OP.is_gt)
                flag = sm.tile([P, 1], f32, tag=f"flag_{tag}")
                nc.vector.tensor_tensor(out=flag, in0=p1o[:, 1:2],
                                        in1=p1o[:, 0:1], op=OP.is_gt)
                neq = sm.tile([P, 1], f32, tag=f"neq_{tag}")
                nc.vector.tensor_tensor(out=neq, in0=cur, in1=flag,
                                        op=OP.not_equal)
                sameflag = sm.tile([P, 1], f32, tag=f"same_{tag}")
                nc.vector.tensor_scalar(out=sameflag, in0=neq, scalar1=-1.0,
                                        scalar2=1.0, op0=OP.mult, op1=OP.add)
                om = sm.tile([P, 1], f32, tag=f"om_{tag}")
                nc.vector.tensor_scalar(out=om, in0=cur, scalar1=-1.0,
                                        scalar2=1.0, op0=OP.mult, op1=OP.add)
                cond = sm.tile([P, 1], f32, tag=f"cond_{tag}")
                nc.vector.tensor_mul(cond, neq, om)
                nc.vector.tensor_mul(cond, cond, yf)
                p1m1 = sm.tile([P, 1], f32, tag=f"p1m1_{tag}")
                nc.vector.tensor_scalar_add(p1m1, prob1, -1.0)
                wv = sm.tile([P, 1], f32, tag=f"wv_{tag}")
                nc.vector.tensor_mul(wv, cond, p1m1)
                nc.vector.tensor_scalar_add(wv, wv, 1.0)
                return wv, sameflag, ls1, d1

            def sigmoid_weight_chain(p1, p1o, yf, tag):
                """Same w/same as weight_chain but prob1 = sigmoid(d) via DVE
                reciprocal: one ACT hop instead of the 3-hop lse chain."""
                d1 = sm.tile([P, 1], f32, tag=f"d1_{tag}")
                nc.vector.tensor_sub(d1, p1[:, 1:2], p1[:, 0:1])
                nd = sm.tile([P, 1], f32, tag=f"nd_{tag}")
                nc.vector.tensor_scalar_mul(nd, d1, -1.0)
                prob1 = sm.tile([P, 1], f32, tag=f"pr_{tag}")
                nc.scalar.activation(out=prob1, in_=nd, func=AF.Exp)
                nc.vector.tensor_scalar_add(prob1, prob1, 1.0)
                nc.vector.reciprocal(prob1, prob1)
                cur = sm.tile([P, 1], f32, tag=f"cur_{tag}")
                nc.vector.tensor_tensor(out=cur, in0=p1[:, 1:2],
                                        in1=p1[:, 0:1], op=OP.is_gt)
                flag = sm.tile([P, 1], f32, tag=f"flag_{tag}")
                nc.vector.tensor_tensor(out=flag, in0=p1o[:, 1:2],
                                        in1=p1o[:, 0:1], op=OP.is_gt)
                neq = sm.tile([P, 1], f32, tag=f"neq_{tag}")
                nc.vector.tensor_tensor(out=neq, in0=cur, in1=flag,
                                        op=OP.not_equal)
                sameflag = sm.tile([P, 1], f32, tag=f"same_{tag}")
                nc.vector.tensor_scalar(out=sameflag, in0=neq, scalar1=-1.0,
                                        scalar2=1.0, op0=OP.mult, op1=OP.add)
                om = sm.tile([P, 1], f32, tag=f"om_{tag}")
                nc.vector.tensor_scalar(out=om, in0=cur, scalar1=-1.0,
                                        scalar2=1.0, op0=OP.mult, op1=OP.add)
                cond = sm.tile([P, 1], f32, tag=f"cond_{tag}")
                nc.vector.tensor_mul(cond, neq, om)
                nc.vector.tensor_mul(cond, cond, yf)
                p1m1 = sm.tile([P, 1], f32, tag=f"p1m1_{tag}")
                nc.vector.tensor_scalar_add(p1m1, prob1, -1.0)
                wv = sm.tile([P, 1], f32, tag=f"wv_{tag}")
                nc.vector.tensor_mul(wv, cond, p1m1)
                nc.vector.tensor_scalar_add(wv, wv, 1.0)
                return wv, sameflag

            # ---- CAM-path coefficients (emitted FIRST: the chunk matmuls
            # need them; short sigmoid chain, ready by the time chunk0 lands)
            if masked:
                yfc = smc[:, 8:9]
                wc, samec = sigmoid_weight_chain(smc[:, 0:2], smc[:, 2:4],
                                                 yfc, "cam")
            else:
                yfc = smt[:, 8:9]
                wc, samec = sigmoid_weight_chain(smt[:, 0:2], smt[:, 2:4],
                                                 yfc, "camf")
            coef_er = sm.tile([P, 1], f32)    # w*yf/(B*HW)
            nc.vector.scalar_tensor_tensor(out=coef_er, in0=wc,
                                           scalar=1.0 / (B * HW), in1=yfc,
                                           op0=OP.mult, op1=OP.mult)
            coef_sp = sm.tile([P, 1], f32)    # yf*same/(B*HW)
            nc.vector.scalar_tensor_tensor(out=coef_sp, in0=samec,
                                           scalar=1.0 / (B * HW), in1=yfc,
                                           op0=OP.mult, op1=OP.mult)

            # ---- CE path as a generator: per-sample losses for this core's
            # 64 batches, interleaved into per-chunk DVE slack ----
            cepart = sm.tile([P, 1], f32)     # w*(ce+ce_back)/(2B) per half-row

            def ce_chain():
                p1 = smt[:, 0:2]
                p2 = smt[:, 4:6]
                pb = smt[:, 6:8]
                yf = smt[:, 8:9]
                wv, _, ls1, d1 = weight_chain(p1, smt[:, 2:4], yf, "ce")
                yield
                ls2_, d2 = lse2(p2, "p2")
                yield
                lsb, _ = lse2(pb, "pb")
                yield
                sel1 = sm.tile([P, 1], f32)
                nc.vector.tensor_mul(sel1, yf, d1)
                nc.vector.tensor_add(sel1, p1[:, 0:1], sel1)
                ce1 = sm.tile([P, 1], f32)
                nc.vector.tensor_sub(ce1, ls1, sel1)
                yield
                sel2 = sm.tile([P, 1], f32)
                nc.vector.tensor_mul(sel2, yf, d2)
                nc.vector.tensor_add(sel2, p2[:, 0:1], sel2)
                ce2 = sm.tile([P, 1], f32)
                nc.vector.tensor_sub(ce2, ls2_, sel2)
                yield
                q = sm.tile([P, 1], f32)      # q = 2*(ce + ce_back)
                nc.vector.tensor_add(q, ce1, ce2)
                cebr = sm.tile([P, 1], f32)
                nc.vector.tensor_sub(cebr, lsb, pb[:, 0:1])
                nc.vector.tensor_mul(cebr, cebr, yf)
                nc.vector.tensor_add(q, q, cebr)
                yield
                nc.vector.scalar_tensor_tensor(out=cepart, in0=q,
                                               scalar=1.0 / (4 * B), in1=wv,
                                               op0=OP.mult, op1=OP.mult)

            ce_steps = ce_chain()
            pt = ps.tile([1, 1], f32)

            # ---- heavy streaming part ----
            off = 0
            for ci, cf in enumerate(chunks):
                last = ci == len(chunks) - 1
                abct = big.tile([P, 3 * cf], f32, tag="abct")
                nc.sync.dma_start(out=abct, in_=abc[:, 3 * off:3 * (off + cf)])
                off += cf
                at = abct[:, 0:cf]
                bt = abct[:, cf:2 * cf]
                ct = abct[:, 2 * cf:3 * cf]
                d = big.tile([P, cf], f32, tag="d")
                nc.vector.tensor_sub(d, at, bt)
                if last:
                    # keep the tail off the congested ACT queue: DVE fused
                    # square+row-sum (custom uop, no accumulator-read step)
                    nc.vector.affine_mul_reduce(
                        out=d, accum_out=er_parts[:, ci:ci + 1],
                        in0=d, in1=d, scale=1.0, bias=0.0)
                else:
                    nc.scalar.activation(out=d, in_=d, func=AF.Square,
                                         accum_out=er_parts[:, ci:ci + 1])
                nc.tensor.matmul(out=pt, lhsT=coef_er,
                                 rhs=er_parts[:, ci:ci + 1], start=(ci == 0),
                                 stop=False)
                e = big.tile([P, cf], f32, tag="e")
                nc.vector.tensor_sub(e, at, ct)
                if last:
                    nc.vector.affine_mul_reduce(
                        out=e, accum_out=sp_parts[:, ci:ci + 1],
                        in0=e, in1=e, scale=1.0, bias=0.0)
                else:
                    nc.scalar.activation(out=e, in_=e, func=AF.Square,
                                         accum_out=sp_parts[:, ci:ci + 1])
                nc.tensor.matmul(out=pt, lhsT=coef_sp,
                                 rhs=sp_parts[:, ci:ci + 1], start=False,
                                 stop=False)
                next(ce_steps, None)

            # drain remaining CE steps, then fold the cepart term in last
            for _ in ce_steps:
                pass
            nc.tensor.matmul(out=pt, lhsT=cepart, rhs=ones, start=False,
                             stop=True)

            res_sb = sm.tile([1, 1], f32)
            nc.vector.tensor_copy(res_sb, pt)
            nc.sync.dma_start(out=outp, in_=res_sb)

    nc.compile()
    return nc


def _get_nc(masked):
    key = "mask" if masked else "full"
    if key not in _NC_CACHE:
        _NC_CACHE[key] = _build_nc(masked)
    return _NC_CACHE[key]


def _interleave(a, b, c, chunks):
    """[P, row] x3 -> [P, 3*row] with a/b/c interleaved per chunk."""
    row = a.shape[1]
    abc = np.empty((P, 3 * row), dtype=np.float32)
    off = 0
    for cf in chunks:
        sl = slice(off, off + cf)
        abc[:, 3 * off:3 * off + cf] = a[:, sl]
        abc[:, 3 * off + cf:3 * off + 2 * cf] = b[:, sl]
        abc[:, 3 * off + 2 * cf:3 * off + 3 * cf] = c[:, sl]
        off += cf
    return abc


def kernel(preds1, cams1, preds1_back, preds2, cams2, y, index):
    from concourse.bass_utils import run_bass_kernel_spmd

    idx = int(np.asarray(index))
    preds1 = np.asarray(preds1, dtype=np.float32)
    preds1_back = np.asarray(preds1_back, dtype=np.float32)
    preds2 = np.asarray(preds2, dtype=np.float32)
    cams1 = np.asarray(cams1, dtype=np.float32)
    cams2 = np.asarray(cams2, dtype=np.float32)
    yi = np.asarray(y).astype(np.int64).reshape(B)
    yf = yi.astype(np.float32).reshape(B, 1)

    sel = np.flatnonzero(yi == 1)
    masked = len(sel) <= CAP
    nc = _get_nc(masked)

    in_maps = []
    for k in range(NCORES):
        s = slice(k * BPC, (k + 1) * BPC)
        sm_host = np.concatenate(
            [preds1[idx, s], preds1[1 - idx, s], preds2[idx, s],
             preds1_back[idx, s], yf[s]], axis=1)          # [64, 9]
        im = {"small": np.ascontiguousarray(
            np.repeat(sm_host, 2, axis=0))}                # [128, 9]

        if masked:
            sel_k = sel[k * SLOTS:(k + 1) * SLOTS]
            nk = len(sel_k)
            a = np.zeros((SLOTS, HW), dtype=np.float32)
            b = np.zeros((SLOTS, HW), dtype=np.float32)
            c = np.zeros((SLOTS, HW), dtype=np.float32)
            a[:nk] = cams1[idx, sel_k, 1].reshape(nk, HW)
            b[:nk] = cams2[idx, sel_k, 1].reshape(nk, HW)
            c[:nk] = cams1[1 - idx, sel_k, 1].reshape(nk, HW)
            im["abc"] = _interleave(a.reshape(P, QROW), b.reshape(P, QROW),
                                    c.reshape(P, QROW), CHUNKS_MASK)
            sc = np.zeros((SLOTS, 9), dtype=np.float32)
            sc[:nk] = np.concatenate(
                [preds1[idx, sel_k], preds1[1 - idx, sel_k],
                 preds2[idx, sel_k], preds1_back[idx, sel_k],
                 yf[sel_k]], axis=1)
            im["small_cam"] = np.ascontiguousarray(np.repeat(sc, 4, axis=0))
        else:
            a = cams1[idx, s, 1].reshape(P, HALF)
            b = cams2[idx, s, 1].reshape(P, HALF)
            c = cams1[1 - idx, s, 1].reshape(P, HALF)
            im["abc"] = _interleave(a, b, c, CHUNKS_FULL)
        in_maps.append(im)

    trace = bool(int(os.environ.get("KERNEL_TRACE", "0")))
    res = run_bass_kernel_spmd(nc, in_maps, core_ids=list(range(NCORES)),
                               trace=trace)
    kernel.last_exec_time_ns = res.exec_time_ns
    total = sum(float(res.results[k]["out"][0, 0]) for k in range(NCORES))
    return np.array(total, dtype=np.float32)


kernel.last_exec_time_ns = None
